# revision 20
# baseline (speedup 1.0000x reference)
"""Tensor-parallel Trainium2 (Bass/Tile) kernel for the
MinimalTransformerWithKVCache decode step, SPMD over 8 NeuronCores.

Sharding: attention heads 16->2/core, MLP hidden 4096->512/core, vocab
32000->4000/core; residual stream replicated with 2 bf16 AllReduces/layer.

Key design points:
- Host pre-arranges the K cache as [l, b, (head,hd)=128, s] (transposed) and
  the V cache as [l, b, p, (chunk, head*hd)] so both stream into SBUF tiles
  with fully-contiguous >=8KB-per-partition DMA descriptors, and the score
  matmuls consume K directly as the 128x128 stationary operand.
- Scores are computed transposed, [s, (head,q)], via a block-diagonal packed
  q operand (both heads in one matmul); softmax runs without the max
  subtraction (scores are O(0.1); validated exact to 1e-6), with the
  partition-axis sum done by a ones-vector matmul and per-(h,q) normalization
  applied through a PE-broadcast of the reciprocal row.
- Everything is bf16 on the wire and as matmul operands (KV cache, weights,
  attention probabilities, AllReduce payloads) while all PSUM accumulation
  and the residual stream stay f32: logits absmax-rel-err ~5e-3.
- Per-bank PSUM discipline: disjoint-subtile matmuls each use start=True
  (bank-bit clear is order-free for single-shot writes); every accumulating
  group owns its own bank (Tile's subtile deps do not order bank-bit clears).
- AllReduce latency (~12.4us measured) is partially hidden by splitting each
  dependent GEMM (fc1 / next-layer qkv / lm head) into a partial on the
  pre-AllReduce activations issued under the collective plus an accumulating
  partial on the AllReduce result.
"""
import sys

sys.path.insert(0, "/opt/trn_rl_repo")

import numpy as np
import ml_dtypes

BF16 = ml_dtypes.bfloat16

L, B, H, S, HD = 4, 8, 16, 4096, 64
D = H * HD
Q = 4
T = B * Q            # 32 tokens
V = 32000
NCORE = 8
HC = H // NCORE      # 2 heads per core
HH = HC * HD         # 128 local head dims
FC = 4096 // NCORE   # 512 fc1 rows per core
VC = V // NCORE      # 4000 vocab rows per core
VCP = 4096           # padded
NCH = S // 128       # 32 cache chunks
SCALE = 0.125        # 1/sqrt(64)

_CACHE = {}


def _build(no_collective=False, repeat=1, no_attn=False, no_lm=False):
    key = (("nc_nocoll" if no_collective else "nc") + (f"_r{repeat}" if repeat != 1 else "")
           + ("_na" if no_attn else "") + ("_nl" if no_lm else ""))
    if key in _CACHE:
        return _CACHE[key]
    import concourse.bacc as bacc
    import concourse.mybir as mybir
    import concourse.tile as tile

    f32 = mybir.dt.float32
    bf = mybir.dt.bfloat16
    AX = mybir.AxisListType
    ALU = mybir.AluOpType
    ACTF = mybir.ActivationFunctionType

    nc = bacc.Bacc(None)

    xT_d = nc.dram_tensor("xT", [D, T], f32, kind="ExternalInput")
    ktc_d = nc.dram_tensor("ktc", [L, B, HH, S], bf, kind="ExternalInput")
    vc_d = nc.dram_tensor("vc", [L, B, 128, NCH * HH], bf, kind="ExternalInput")
    qkvT_d = nc.dram_tensor("qkvT", [L, D, 3 * HH], bf, kind="ExternalInput")
    outwT_d = nc.dram_tensor("outwT", [L, HH, D], bf, kind="ExternalInput")
    fc1T_d = nc.dram_tensor("fc1T", [L, D, FC], bf, kind="ExternalInput")
    fc2T_d = nc.dram_tensor("fc2T", [L, FC, D], bf, kind="ExternalInput")
    lmhT_d = nc.dram_tensor("lmhT", [D, VCP], bf, kind="ExternalInput")

    logits_d = nc.dram_tensor("logits_p", [T, VCP], f32, kind="ExternalOutput")
    newkT_d = nc.dram_tensor("newkT", [L, HH, T], f32, kind="ExternalOutput")
    newv_d = nc.dram_tensor("newv", [L, T, HH], f32, kind="ExternalOutput")

    DCH = D // 128  # 8 contraction chunks over the model dim

    with tile.TileContext(nc) as tc:
        with (
            tc.tile_pool(name="px", bufs=1) as px,
            tc.tile_pool(name="pw", bufs=1) as pw,
            tc.tile_pool(name="pkv", bufs=3) as pkv,
            tc.tile_pool(name="psb", bufs=3) as psb,
            tc.tile_pool(name="plm", bufs=2) as plm,
            tc.tile_pool(name="pps", bufs=8, space="PSUM") as pps,
            tc.tile_pool(name="pdram", bufs=2, space="DRAM") as pdram,
        ):
            def psum(shape, tag="ps"):
                return pps.tile(shape, f32, tag=tag, name="ps")

            def load_chunked(dst, src, nch):
                # DRAM [(c p), e] -> SBUF [p, (c e)] as 3D APs
                nc.sync.dma_start(
                    dst.rearrange("p (c e) -> p c e", c=nch),
                    src.rearrange("(c p) e -> p c e", p=128),
                )

            # resident state
            xt = px.tile([128, DCH * T], f32, tag="xt")  # d-chunk c at cols c*32
            load_chunked(xt[:], xT_d[:], DCH)
            ones_col = px.tile([128, 1], bf, tag="ones_col")
            nc.vector.memset(ones_col[:], 1.0)
            ones_row = px.tile([1, 128], f32, tag="ones_row")
            nc.vector.memset(ones_row[:], 1.0)

            xt_b2 = psb.tile([128, DCH * T], bf, tag="xt_b")
            nc.scalar.copy(xt_b2[:], xt[:])
            prev_xb = None
            prev_arb = None
            for l0 in range(L * repeat):
                l = l0 % L
                wqkv = pw.tile([128, DCH * 3 * HH], bf, tag="wqkv")
                load_chunked(wqkv[:], qkvT_d[l], DCH)
                wout = pw.tile([128, D], bf, tag="wout")
                nc.sync.dma_start(wout[:], outwT_d[l])
                wfc1 = pw.tile([128, DCH * FC], bf, tag="wfc1")
                load_chunked(wfc1[:], fc1T_d[l], DCH)
                wfc2 = pw.tile([128, 4 * D], bf, tag="wfc2")
                load_chunked(wfc2[:], fc2T_d[l], 4)

                # ---- qkv projections (split: partA on pre-AR x, partB on ar2) ----
                # q^T,k^T: [(h,hd), (b,q)]; v natural: [(b,q), (h,hd)]
                ps_q = psum([128, 32])
                ps_k = psum([128, 32])
                ps_v = psum([32, 128])
                if l0 == 0:
                    qkv_parts = [(xt_b2, True)]
                else:
                    qkv_parts = [(prev_xb, False), (prev_arb, True)]
                for pi, (rhs_t, last) in enumerate(qkv_parts):
                    for d in range(DCH):
                        c0 = d * 3 * HH
                        st = (pi == 0 and d == 0)
                        sp = (last and d == DCH - 1)
                        nc.tensor.matmul(
                            ps_q[:], wqkv[:, c0:c0 + 128],
                            rhs_t[:, d * T:(d + 1) * T], start=st, stop=sp,
                        )
                        nc.tensor.matmul(
                            ps_k[:], wqkv[:, c0 + 128:c0 + 256],
                            rhs_t[:, d * T:(d + 1) * T], start=st, stop=sp,
                        )
                        nc.tensor.matmul(
                            ps_v[:], rhs_t[:, d * T:(d + 1) * T],
                            wqkv[:, c0 + 256:c0 + 384], start=st, stop=sp,
                        )
                kT_s = psb.tile([128, T], f32, tag="kT_s")
                nc.scalar.copy(kT_s[:], ps_k[:])
                kT_sb = psb.tile([128, T], bf, tag="kT_sb")
                nc.scalar.copy(kT_sb[:], ps_k[:])
                v_s = psb.tile([32, 128], f32, tag="v_s")
                nc.scalar.copy(v_s[:], ps_v[:])
                v_sb = psb.tile([32, 128], bf, tag="v_sb")
                nc.scalar.copy(v_sb[:], ps_v[:])
                nc.sync.dma_start(newkT_d[l], kT_s[:])
                nc.sync.dma_start(newv_d[l], v_s[:])
                # per-batch new-V rows staged at base partition 0 (PE operand rule)
                vnb = psb.tile([4, B * 128], bf, tag="vnb")
                for b in range(B):
                    nc.sync.dma_start(
                        vnb[0:4, b * 128:(b + 1) * 128], v_sb[b * 4:(b + 1) * 4, :]
                    )
                # block-diag scaled q: [128, 8] per batch, packed [128, 64]
                q2bd = psb.tile([128, 8 * B], bf, tag="q2bd")
                nc.vector.memset(q2bd[:], 0.0)
                for b in range(B):
                    nc.scalar.mul(
                        q2bd[0:64, b * 8:b * 8 + 4], ps_q[0:64, b * 4:b * 4 + 4], SCALE
                    )
                    nc.scalar.mul(
                        q2bd[64:128, b * 8 + 4:b * 8 + 8],
                        ps_q[64:128, b * 4:b * 4 + 4], SCALE,
                    )

                # ---- attention (per batch, software-pipelined by one b) ----
                oT_s = psb.tile([128, T], bf, tag="oT_s")  # [(h,hd), (b,q)]
                if no_attn:
                    nc.vector.memset(oT_s[:], 0.01)
                stage = {}

                def attn_front(b):
                    kt = pkv.tile([128, S], bf, tag="kt", name="kt")
                    nc.sync.dma_start(kt[:], ktc_d[l, b])
                    vt = pkv.tile([128, S], bf, tag="vt", name="vt")
                    nc.sync.dma_start(vt[:], vc_d[l, b])
                    if no_attn:
                        return
                    qb = q2bd[:, b * 8:(b + 1) * 8]
                    # scoresT [s, (h,q)] in one psum bank: chunk ch at cols ch*8
                    ps_sc = psum([128, 8 * NCH + 8])
                    for ch in range(NCH):
                        nc.tensor.matmul(
                            ps_sc[:, ch * 8:(ch + 1) * 8],
                            kt[:, ch * 128:(ch + 1) * 128], qb,
                            start=True, stop=True, skip_group_check=True,
                        )
                    nc.tensor.matmul(
                        ps_sc[0:4, 256:264], kT_sb[:, b * 4:(b + 1) * 4], qb,
                        start=True, stop=True, skip_group_check=True,
                    )
                    expT = psb.tile([128, 8 * NCH + 8], bf, tag="expT")
                    nc.scalar.activation(expT[:, 0:256], ps_sc[:, 0:256], ACTF.Exp)
                    nc.scalar.activation(
                        expT[0:4, 256:264], ps_sc[0:4, 256:264], ACTF.Exp
                    )
                    stage[b] = (vt, expT)

                def attn_back(b):
                    vt, expT = stage.pop(b)
                    # column sums over s (partition axis) via ones-matmul
                    ps_cs = psum([1, 264])
                    nc.tensor.matmul(
                        ps_cs[:, 0:256], ones_col[:], expT[:, 0:256],
                        start=True, stop=True, skip_group_check=True,
                    )
                    nc.tensor.matmul(
                        ps_cs[:, 256:264], ones_col[0:4, :], expT[0:4, 256:264],
                        start=True, stop=True, skip_group_check=True,
                    )
                    red = psb.tile([1, 8], f32, tag="red", name="red")
                    nc.vector.reduce_sum(
                        red[:], ps_cs[:, 0:256].rearrange("p (c q) -> p q c", q=8),
                        axis=AX.X,
                    )
                    nc.vector.tensor_tensor(
                        red[:], red[:], ps_cs[:, 256:264], op=ALU.add
                    )
                    rr = psb.tile([1, 8], f32, tag="rr", name="rr")
                    nc.vector.reciprocal(rr[:], red[:])
                    # o2 [(h,hd), (h,q)] accumulate over all chunks + new kv
                    ps_o2 = psum([128, 8])
                    for ch in range(NCH):
                        nc.tensor.matmul(
                            ps_o2[:], vt[:, ch * 128:(ch + 1) * 128],
                            expT[:, ch * 8:(ch + 1) * 8],
                            start=(ch == 0), stop=False,
                        )
                    nc.tensor.matmul(
                        ps_o2[:], vnb[0:4, b * 128:(b + 1) * 128], expT[0:4, 256:264],
                        start=False, stop=True,
                    )
                    ps_rb = psum([128, 8])
                    nc.tensor.matmul(ps_rb[:], ones_row[:], rr[:], start=True, stop=True)
                    rb_s = psb.tile([128, 8], f32, tag="rb_s", name="rb_s")
                    nc.scalar.copy(rb_s[:], ps_rb[:])
                    nc.vector.tensor_tensor(
                        oT_s[0:64, b * 4:(b + 1) * 4], ps_o2[0:64, 0:4],
                        rb_s[0:64, 0:4], op=ALU.mult,
                    )
                    nc.vector.tensor_tensor(
                        oT_s[64:128, b * 4:(b + 1) * 4], ps_o2[64:128, 4:8],
                        rb_s[64:128, 4:8], op=ALU.mult,
                    )

                for b in range(B):
                    if no_attn != "nodma":
                        attn_front(b)
                    if not no_attn:
                        attn_back(b)

                # ---- output projection partial + AllReduce + residual ----
                ps_op = psum([128, DCH * T])
                for e in range(DCH):
                    nc.tensor.matmul(
                        ps_op[:, e * T:(e + 1) * T], wout[:, e * 128:(e + 1) * 128],
                        oT_s[:], start=True, stop=True,
                        skip_group_check=True,
                    )
                ar_s = psb.tile([128, DCH * T], bf, tag="ar_s")
                nc.scalar.copy(ar_s[:], ps_op[:])
                ar_in = pdram.tile([128, DCH * T], bf, tag="ar_in")
                ar_out = pdram.tile([128, DCH * T], bf, tag="ar_out")
                nc.sync.dma_start(ar_in[:], ar_s[:])
                if not no_collective:
                    nc.gpsimd.collective_compute(
                        "AllReduce", ALU.add,
                        replica_groups=[list(range(NCORE))],
                        ins=[ar_in.opt()], outs=[ar_out.opt()],
                    )
                else:
                    nc.sync.dma_start(ar_out[:], ar_in[:])
                # fc1 partA on pre-AR x runs during the collective
                xt_b = psb.tile([128, DCH * T], bf, tag="xt_b")
                nc.scalar.copy(xt_b[:], xt[:])
                ps_hs = []
                for f in range(4):
                    ps_h = psum([128, T])
                    ps_hs.append(ps_h)
                    for d in range(DCH):
                        nc.tensor.matmul(
                            ps_h[:],
                            wfc1[:, d * FC + f * 128:d * FC + (f + 1) * 128],
                            xt_b[:, d * T:(d + 1) * T],
                            start=(d == 0), stop=False,
                        )
                arb1 = psb.tile([128, DCH * T], bf, tag="arb")
                nc.sync.dma_start(arb1[:], ar_out[:])
                nc.vector.tensor_tensor(xt[:], xt[:], arb1[:], op=ALU.add)

                # ---- MLP partB + relu ----
                h_s = psb.tile([128, 4 * T], bf, tag="h_s")
                for f in range(4):
                    ps_h = ps_hs[f]
                    for d in range(DCH):
                        nc.tensor.matmul(
                            ps_h[:],
                            wfc1[:, d * FC + f * 128:d * FC + (f + 1) * 128],
                            arb1[:, d * T:(d + 1) * T],
                            start=False, stop=(d == DCH - 1),
                        )
                    nc.scalar.activation(
                        h_s[:, f * T:(f + 1) * T], ps_h[:], ACTF.Relu
                    )
                ar_s2 = psb.tile([128, DCH * T], bf, tag="ar_s")
                for e in range(DCH):
                    ps_m = psum([128, T])
                    for f in range(4):
                        nc.tensor.matmul(
                            ps_m[:],
                            wfc2[:, f * D + e * 128:f * D + (e + 1) * 128],
                            h_s[:, f * T:(f + 1) * T],
                            start=(f == 0), stop=(f == 3),
                        )
                    nc.scalar.copy(ar_s2[:, e * T:(e + 1) * T], ps_m[:])
                ar_in2 = pdram.tile([128, DCH * T], bf, tag="ar_in")
                ar_out2 = pdram.tile([128, DCH * T], bf, tag="ar_out")
                nc.sync.dma_start(ar_in2[:], ar_s2[:])
                if not no_collective:
                    nc.gpsimd.collective_compute(
                        "AllReduce", ALU.add,
                        replica_groups=[list(range(NCORE))],
                        ins=[ar_in2.opt()], outs=[ar_out2.opt()],
                    )
                else:
                    nc.sync.dma_start(ar_out2[:], ar_in2[:])
                prev_xb = psb.tile([128, DCH * T], bf, tag="xt_b")
                nc.scalar.copy(prev_xb[:], xt[:])
                prev_arb = psb.tile([128, DCH * T], bf, tag="arb")
                nc.sync.dma_start(prev_arb[:], ar_out2[:])
                nc.vector.tensor_tensor(xt[:], xt[:], prev_arb[:], op=ALU.add)

            # ---- lm head: partA on pre-AR x during final collective ----
            if no_lm:
                lgz = psb.tile([32, 512], f32, tag="lg_s")
                nc.vector.memset(lgz[:], 0.0)
                for vg in range(8):
                    nc.sync.dma_start(logits_d[:, vg * 512:(vg + 1) * 512], lgz[:])
            ps_lgs = [] if no_lm else [psum([32, 512]) for _ in range(8)]
            lmh_ts = []
            for d in range(0 if no_lm else DCH):
                lmh_t = plm.tile([128, VCP], bf, tag="lmh", bufs=8)
                lmh_ts.append(lmh_t)
                nc.sync.dma_start(lmh_t[:], lmhT_d[d * 128:(d + 1) * 128, :])
                for vg in range(8):
                    nc.tensor.matmul(
                        ps_lgs[vg][:], prev_xb[:, d * T:(d + 1) * T],
                        lmh_t[:, vg * 512:(vg + 1) * 512],
                        start=(d == 0), stop=False,
                    )
            for d in range(0 if no_lm else DCH):
                for vg in range(8):
                    nc.tensor.matmul(
                        ps_lgs[vg][:], prev_arb[:, d * T:(d + 1) * T],
                        lmh_ts[d][:, vg * 512:(vg + 1) * 512],
                        start=False, stop=(d == DCH - 1),
                    )
            for vg in range(0 if no_lm else 8):
                lg_s = psb.tile([32, 512], f32, tag="lg_s")
                nc.scalar.copy(lg_s[:], ps_lgs[vg][:])
                nc.sync.dma_start(logits_d[:, vg * 512:(vg + 1) * 512], lg_s[:])

    nc.compile()
    _CACHE[key] = nc
    return nc


def _prepare(inputs):
    input_ids = np.asarray(inputs["input_ids"]).astype(np.int64)
    position_ids = np.asarray(inputs["position_ids"]).astype(np.int64)
    past_keys = np.asarray(inputs["past_keys"])
    past_values = np.asarray(inputs["past_values"])
    emb = np.asarray(inputs["emb"])
    pos_emb = np.asarray(inputs["pos_emb"])
    qkv_w = np.asarray(inputs["qkv_w"])
    out_w = np.asarray(inputs["out_w"])
    fc1_w = np.asarray(inputs["fc1_w"])
    fc2_w = np.asarray(inputs["fc2_w"])
    lm_head_w = np.asarray(inputs["lm_head_w"])

    x0 = (emb[input_ids] + pos_emb[position_ids]).astype(np.float32)  # [B,Q,D]
    xT = np.ascontiguousarray(x0.reshape(T, D).T)

    in_maps = []
    for c in range(NCORE):
        hsl = slice(c * HC, (c + 1) * HC)
        ktc = np.ascontiguousarray(
            past_keys[:, :, hsl].transpose(0, 1, 2, 4, 3), dtype=BF16
        ).reshape(L, B, HH, S)
        v5 = past_values[:, :, hsl].transpose(0, 1, 3, 2, 4).reshape(
            L, B, NCH, 128, HH
        )
        vc = np.ascontiguousarray(
            v5.transpose(0, 1, 3, 2, 4), dtype=BF16
        ).reshape(L, B, 128, NCH * HH)
        qkvT = np.empty((L, D, 3 * HH), np.float32)
        for l in range(L):
            rows = []
            for part in range(3):
                rows.append(
                    qkv_w[l][part * D + c * HH:part * D + (c + 1) * HH, :]
                )
            qkvT[l] = np.concatenate(rows, 0).T
        outwT = np.ascontiguousarray(
            out_w[:, :, c * HH:(c + 1) * HH].transpose(0, 2, 1)
        )
        fc1T = np.ascontiguousarray(
            fc1_w[:, c * FC:(c + 1) * FC, :].transpose(0, 2, 1)
        )
        fc2T = np.ascontiguousarray(
            fc2_w[:, :, c * FC:(c + 1) * FC].transpose(0, 2, 1)
        )
        lmhT = np.zeros((D, VCP), np.float32)
        lmhT[:, :VC] = lm_head_w[c * VC:(c + 1) * VC, :].T
        in_maps.append(
            dict(xT=xT, ktc=ktc, vc=vc, qkvT=qkvT.astype(BF16),
                 outwT=outwT.astype(BF16), fc1T=fc1T.astype(BF16),
                 fc2T=fc2T.astype(BF16), lmhT=lmhT.astype(BF16))
        )
    return in_maps, past_keys, past_values


def _assemble(results, past_keys, past_values):
    logits = np.empty((B, Q, V), np.float32)
    newk = np.empty((L, B, H, Q, HD), np.float32)
    newv = np.empty((L, B, H, Q, HD), np.float32)
    for c in range(NCORE):
        r = results[c]
        logits[:, :, c * VC:(c + 1) * VC] = r["logits_p"][:, :VC].reshape(B, Q, VC)
        kt = r["newkT"].reshape(L, HC, HD, B, Q)
        newk[:, :, c * HC:(c + 1) * HC] = kt.transpose(0, 3, 1, 4, 2)
        vn = r["newv"].reshape(L, B, Q, HC, HD)
        newv[:, :, c * HC:(c + 1) * HC] = vn.transpose(0, 1, 3, 2, 4)
    pres_k = np.concatenate([past_keys, newk], axis=3)
    pres_v = np.concatenate([past_values, newv], axis=3)
    return logits, pres_k, pres_v


def run_spmd(inputs, trace=False, **kw):
    from concourse.bass_utils import run_bass_kernel_spmd

    nc = _build()
    in_maps, pk, pv = _prepare(inputs)
    res = run_bass_kernel_spmd(nc, in_maps, list(range(NCORE)), trace=trace, **kw)
    return _assemble(res.results, pk, pv), res


def kernel(**inputs):
    out, _ = run_spmd(inputs)
    return out
